# revision 26
# baseline (speedup 1.0000x reference)
"""Trainium2 Bass kernel for nn_AutoregressiveArithmeticTransformer.

6-layer dense transformer: B=16, T=512, E=512, NH=8 heads x HS=64, FF=2048,
V=16, causal attention, pre-LN, learned abacus embedding, logits / 0.8.

Strategy: data-parallel over batch across 8 NeuronCores (2 sequences per
core, no collectives). Activations live feature-major in SBUF
([E-partitions, tokens]); weights are streamed per-layer in bf16; all
matmuls run in bf16 with fp32 PSUM accumulation; the residual stream stays
fp32. LayerNorm statistics are computed with ones-matmuls on the PE;
attention scores are computed transposed ([tk, tq]) so the softmax
denominator is also a ones-matmul; V is produced token-major directly so
no transposes are ever needed.

All ops are token-tile (512) granular so the two sequences per core form
independent dependency streams the Tile scheduler can interleave.
"""

import numpy as np
import ml_dtypes

import concourse.bacc as bacc
import concourse.tile as tile
from concourse import mybir

F32 = mybir.dt.float32
BF16 = mybir.dt.bfloat16
AF = mybir.ActivationFunctionType
OP = mybir.AluOpType

# Model constants (hardcoded per contest contract)
V, E, NH, HS, FF, NB, L = 16, 512, 8, 64, 2048, 6, 512
B, T = 16, 512
TEMP = 1.0 * 0.8
EPS = 1e-5
SCALE = HS ** -0.5  # 0.125

NCORES = 8
SEQ = 2              # sequences per core
NTOK = SEQ * T       # 1024 tokens per core
C = E // 128         # 4 E-chunks
CF = FF // 128       # 16 FF-chunks
HP = NH // 2         # 4 head-pairs
NJ = T // 128        # 4 tk chunks per sequence

_PROGRAM_CACHE = {}


def _emit_ln_tt(nc, pools, h_t, ones_t, eps2_t, g_ap, b_ap, trivial, tt,
                hb, sq, xn):
    """One token-tile of LayerNorm into caller-allocated hb/sq/xn tiles."""
    stats, stats_bf = pools["stats"], pools["stats_bf"]
    ps1 = pools["ps1"]
    sl = slice(tt * 512, tt * 512 + 512)
    s1 = ps1.tile([128, 512], F32, tag="ps1")
    s2 = ps1.tile([128, 512], F32, tag="ps1")
    for c in range(C):
        nc.scalar.copy(hb[:, c, sl], h_t[:, c, sl])
        nc.vector.tensor_tensor(sq[:, c, sl], hb[:, c, sl], hb[:, c, sl],
                                OP.mult)
        nc.tensor.matmul(s1[:], ones_t[:], hb[:, c, sl],
                         start=(c == 0), stop=(c == C - 1))
        nc.tensor.matmul(s2[:], ones_t[:], sq[:, c, sl],
                         start=(c == 0), stop=(c == C - 1))
    msq = stats.tile([128, 512], F32, tag="stats")
    nc.scalar.square(msq[:], s1[:])
    var = stats.tile([128, 512], F32, tag="stats")
    nc.vector.scalar_tensor_tensor(out=var[:], in0=s2[:], scalar=float(E),
                                   in1=msq[:], op0=OP.mult, op1=OP.subtract)
    std = stats.tile([128, 512], F32, tag="stats")
    nc.scalar.activation(std[:], var[:], AF.Sqrt, bias=eps2_t[:])
    rc = stats.tile([128, 512], F32, tag="stats")
    nc.vector.reciprocal_approx_fast(out=rc[:], in_=std[:])
    r_bf = stats_bf.tile([128, 512], BF16, tag="r_bf")
    nc.scalar.mul(r_bf[:], rc[:], float(E))
    z_bf = stats_bf.tile([128, 512], BF16, tag="z_bf")
    nc.vector.tensor_tensor(z_bf[:], s1[:], rc[:], OP.mult)
    for c in range(C):
        nc.vector.tensor_tensor(xn[:, c, sl], hb[:, c, sl], r_bf[:], OP.mult)
        nc.vector.tensor_tensor(xn[:, c, sl], xn[:, c, sl], z_bf[:],
                                OP.subtract)
        if not trivial:
            nc.vector.tensor_scalar(out=xn[:, c, sl], in0=xn[:, c, sl],
                                    scalar1=g_ap[:, c:c + 1],
                                    scalar2=b_ap[:, c:c + 1],
                                    op0=OP.mult, op1=OP.add)


def _alloc_ln(pools):
    hb = pools["scr"].tile([128, C, NTOK], BF16, tag="scratch", name="hb")
    sq = pools["scr2"].tile([128, C, NTOK], BF16, tag="sq", name="sq")
    xn = pools["scr"].tile([128, C, NTOK], BF16, tag="scratch", name="xnt")
    return hb, sq, xn


def _emit_ln(nc, pools, h_t, ones_t, eps2_t, g_ap, b_ap, trivial):
    hb, sq, xn = _alloc_ln(pools)
    for tt in range(2):
        _emit_ln_tt(nc, pools, h_t, ones_t, eps2_t, g_ap, b_ap, trivial, tt,
                    hb, sq, xn)
    return xn


def build_program(ln_trivial, nb_run=NB, ln_general_params=True):
    """Build the Bass program. ln_trivial: list of NB*2+1 bools (ln1/ln2 per
    layer then lnf) -- when True the g/b application op is skipped."""
    nc = bacc.Bacc(None, target_bir_lowering=False)

    h0_d = nc.dram_tensor("h0", [128, C * NTOK], F32, kind="ExternalInput")
    wq_d = nc.dram_tensor("wq", [NB, 128, C * 512], BF16, kind="ExternalInput")
    wk_d = nc.dram_tensor("wk", [NB, 128, C * 512], BF16, kind="ExternalInput")
    wv_d = nc.dram_tensor("wv", [NB, 128, C * 512], BF16, kind="ExternalInput")
    pw_d = nc.dram_tensor("pw", [NB, 128, C * 512], BF16, kind="ExternalInput")
    f1_d = nc.dram_tensor("f1", [NB, 128, C * FF], BF16, kind="ExternalInput")
    f2_d = nc.dram_tensor("f2", [NB, 128, CF * 512], BF16, kind="ExternalInput")
    pb_d = nc.dram_tensor("pb", [128, NB * C], F32, kind="ExternalInput")
    fb1_d = nc.dram_tensor("fb1", [128, NB * CF], F32, kind="ExternalInput")
    fb2_d = nc.dram_tensor("fb2", [128, NB * C], F32, kind="ExternalInput")
    ow_d = nc.dram_tensor("ow", [128, C * V], BF16, kind="ExternalInput")
    ob_d = nc.dram_tensor("ob", [V, 1], F32, kind="ExternalInput")
    tri_d = nc.dram_tensor("tri", [128, 128], BF16, kind="ExternalInput")
    lng_d = lnb_d = None
    if ln_general_params:
        lng_d = nc.dram_tensor("lng", [128, (2 * NB + 1) * C], F32,
                               kind="ExternalInput")
        lnb_d = nc.dram_tensor("lnb", [128, (2 * NB + 1) * C], F32,
                               kind="ExternalInput")
    out_d = nc.dram_tensor("logits", [V, NTOK], F32, kind="ExternalOutput")

    from contextlib import ExitStack
    with ExitStack() as ctx:
        tc = ctx.enter_context(tile.TileContext(nc))
        consts = ctx.enter_context(tc.tile_pool(name="consts", bufs=1))
        hpool = ctx.enter_context(tc.tile_pool(name="hpool", bufs=1))
        wqkv = ctx.enter_context(tc.tile_pool(name="wqkv", bufs=1))
        wff1 = ctx.enter_context(tc.tile_pool(name="wff1", bufs=1))
        wff2 = ctx.enter_context(tc.tile_pool(name="wff2", bufs=1))
        scr = ctx.enter_context(tc.tile_pool(name="scr", bufs=4))
        scr2 = ctx.enter_context(tc.tile_pool(name="scr2", bufs=1))
        qk = ctx.enter_context(tc.tile_pool(name="qk", bufs=3))
        vt = ctx.enter_context(tc.tile_pool(name="vt", bufs=2))
        pp = ctx.enter_context(tc.tile_pool(name="pp", bufs=3))
        osb = ctx.enter_context(tc.tile_pool(name="osb", bufs=1))
        ffa = ctx.enter_context(tc.tile_pool(name="ffa", bufs=2))
        stats = ctx.enter_context(tc.tile_pool(name="stats", bufs=6))
        stats_bf = ctx.enter_context(tc.tile_pool(name="stats_bf", bufs=2))
        ps1 = ctx.enter_context(tc.tile_pool(name="ps1", bufs=8, space="PSUM"))
        ps2 = ps1

        pools = {"scr": scr, "scr2": scr2, "stats": stats,
                 "stats_bf": stats_bf, "ps2": ps2, "ps1": ps1}

        ones_t = consts.tile([128, 128], BF16)
        nc.gpsimd.memset(ones_t[:], 1.0)
        eps2_t = consts.tile([128, 1], F32)
        nc.gpsimd.memset(eps2_t[:], float(E) * float(E) * EPS)
        tri_t = consts.tile([128, 128], BF16)
        nc.sync.dma_start(tri_t[:], tri_d[:])
        pb_t = consts.tile([128, NB * C], F32)
        nc.sync.dma_start(pb_t[:], pb_d[:])
        fb1_t = consts.tile([128, NB * CF], F32)
        nc.sync.dma_start(fb1_t[:], fb1_d[:])
        fb2_t = consts.tile([128, NB * C], F32)
        nc.sync.dma_start(fb2_t[:], fb2_d[:])
        ow_t = consts.tile([128, C, V], BF16)
        nc.sync.dma_start(ow_t[:], ow_d[:].rearrange("p (c v) -> p c v", v=V))
        ob_t = consts.tile([V, 1], F32)
        nc.sync.dma_start(ob_t[:], ob_d[:])
        lng_t = lnb_t = None
        if ln_general_params:
            lng_t = consts.tile([128, 2 * NB + 1, C], F32)
            nc.sync.dma_start(lng_t[:], lng_d[:].rearrange(
                "p (l c) -> p l c", c=C))
            lnb_t = consts.tile([128, 2 * NB + 1, C], F32)
            nc.sync.dma_start(lnb_t[:], lnb_d[:].rearrange(
                "p (l c) -> p l c", c=C))

        h_t = hpool.tile([128, C, NTOK], F32)
        nc.sync.dma_start(h_t[:], h0_d[:].rearrange(
            "p (c t) -> p c t", t=NTOK))

        def ln_params(idx):
            if ln_general_params and not ln_trivial[idx]:
                return lng_t[:, idx, :], lnb_t[:, idx, :], False
            return None, None, True

        for i in range(nb_run):
            # ---- load this layer's weights ----
            wq_t = wqkv.tile([128, C, 512], BF16, tag="wq")
            nc.sync.dma_start(wq_t[:], wq_d[i].rearrange(
                "p (c m) -> p c m", m=512))
            wk_t = wqkv.tile([128, C, 512], BF16, tag="wk")
            nc.sync.dma_start(wk_t[:], wk_d[i].rearrange(
                "p (c m) -> p c m", m=512))
            wv_t = wqkv.tile([128, C, 512], BF16, tag="wv")
            nc.sync.dma_start(wv_t[:], wv_d[i].rearrange(
                "p (c m) -> p c m", m=512))
            pw_t = wqkv.tile([128, C, 512], BF16, tag="pw")
            nc.sync.dma_start(pw_t[:], pw_d[i].rearrange(
                "p (c m) -> p c m", m=512))
            f1_t = wff1.tile([128, C, FF], BF16, tag="f1")
            nc.sync.dma_start(f1_t[:], f1_d[i].rearrange(
                "p (c m) -> p c m", m=FF))
            f2_t = wff2.tile([128, CF, 512], BF16, tag="f2")
            nc.sync.dma_start(f2_t[:], f2_d[i].rearrange(
                "p (c m) -> p c m", m=512))

            # ---- LN1 (layer 0: pre-peeled below; others peeled into
            #      the previous layer's FFN emission) ----
            if i == 0:
                g_ap, b_ap, triv = ln_params(0)
                xn = _emit_ln(nc, pools, h_t, ones_t, eps2_t, g_ap, b_ap,
                              triv)
            else:
                xn = xn_next

            # ---- attention, sequence-major with 1-deep pipeline;
            #      proj(tt) emitted right after its sequence's flush so the
            #      dense projection matmuls backfill the other sequence's
            #      attention stalls ----
            o_t = osb.tile([128, C, NTOK], BF16, tag="o")

            def emit_den_o(s, hp, p_t, vt_s):
                base = s * T
                rdens = []
                for h2 in range(2):
                    den = ps1.tile([128, 512], F32, tag="ps1")
                    for j in range(NJ):
                        off = j * 128
                        njw = T - off
                        nc.tensor.matmul(den[:, off:512], ones_t[:],
                                         p_t[:, h2, j, 0:njw],
                                         start=(j == 0), stop=(j == NJ - 1))
                    rd = stats.tile([128, 512], F32, tag="stats")
                    nc.vector.reciprocal_approx_fast(out=rd[:], in_=den[:])
                    rdens.append(rd)
                op_ps = ps1.tile([128, 512], F32, tag="ps1")
                for h2 in range(2):
                    head = hp * 2 + h2
                    for j in range(NJ):
                        off = j * 128
                        njw = T - off
                        nc.tensor.matmul(
                            op_ps[h2 * 64:h2 * 64 + 64, off:T],
                            vt_s[:, j, head * 64:head * 64 + 64],
                            p_t[:, h2, j, 0:njw],
                            start=(j == 0), stop=(j == NJ - 1))
                for h2 in range(2):
                    dsl = slice(h2 * 64, h2 * 64 + 64)
                    nc.vector.tensor_tensor(
                        o_t[dsl, hp, base:base + T], op_ps[dsl, 0:T],
                        rdens[h2][dsl, :], OP.mult)

            def emit_proj(tt):
                sl = slice(tt * 512, tt * 512 + 512)
                for mc in range(C):
                    pj = ps1.tile([128, 512], F32, tag="ps1")
                    for c in range(C):
                        nc.tensor.matmul(pj[:],
                                         pw_t[:, c, mc * 128:(mc + 1) * 128],
                                         o_t[:, c, sl],
                                         start=(c == 0), stop=(c == C - 1))
                    nc.vector.scalar_tensor_tensor(
                        out=h_t[:, mc, sl], in0=pj[:],
                        scalar=pb_t[:, i * C + mc:i * C + mc + 1],
                        in1=h_t[:, mc, sl], op0=OP.add, op1=OP.add)

            for s in range(SEQ):
                ssl = slice(s * 512, s * 512 + 512)
                # token-major V for this sequence
                vt_s = vt.tile([128, NJ, 512], BF16, tag="vt")
                for jg in range(NJ):
                    vp = ps1.tile([128, 512], F32, tag="ps1")
                    for c in range(C):
                        nc.tensor.matmul(
                            vp[:],
                            xn[:, c, s * 512 + jg * 128:s * 512 + (jg + 1) * 128],
                            wv_t[:, c, :],
                            start=(c == 0), stop=(c == C - 1))
                    nc.scalar.copy(vt_s[:, jg, :], vp[:])

                pending = None
                for hp in range(HP):
                    msl = slice(hp * 128, (hp + 1) * 128)
                    q_t = qk.tile([128, 512], BF16, tag="q")
                    k_t = qk.tile([128, 512], BF16, tag="k")
                    qp = ps1.tile([128, 512], F32, tag="ps1")
                    kp = ps1.tile([128, 512], F32, tag="ps1")
                    for c in range(C):
                        nc.tensor.matmul(qp[:], wq_t[:, c, msl], xn[:, c, ssl],
                                         start=(c == 0), stop=(c == C - 1))
                        nc.tensor.matmul(kp[:], wk_t[:, c, msl], xn[:, c, ssl],
                                         start=(c == 0), stop=(c == C - 1))
                    nc.scalar.copy(q_t[:], qp[:])
                    nc.scalar.copy(k_t[:], kp[:])

                    p_t = pp.tile([128, 2, NJ, 512], BF16, tag="p")
                    for j in range(NJ):
                        off = j * 128
                        njw = T - off
                        for h2 in range(2):
                            dsl = slice(h2 * 64, h2 * 64 + 64)
                            sT = ps1.tile([128, 512], F32, tag="ps1")
                            nc.tensor.matmul(
                                sT[:, 0:njw],
                                k_t[dsl, off:off + 128],
                                q_t[dsl, off:T],
                                start=True, stop=True)
                            nc.scalar.activation(
                                p_t[:, h2, j, 0:njw], sT[:, 0:njw],
                                AF.Exp, scale=SCALE)
                        nc.vector.tensor_tensor(
                            p_t[:, :, j, 0:128], p_t[:, :, j, 0:128],
                            tri_t[:, None, :].to_broadcast(
                                (128, 2, 128)), OP.mult)
                    if pending is not None:
                        emit_den_o(*pending)
                    pending = (s, hp, p_t, vt_s)
                emit_den_o(*pending)
                emit_proj(s)

            # ---- LN2 + FFN (token-tile split) ----
            g_ap, b_ap, triv = ln_params(2 * i + 1)
            xn2 = _emit_ln(nc, pools, h_t, ones_t, eps2_t, g_ap, b_ap, triv)

            for tt in range(2):
                sl = slice(tt * 512, tt * 512 + 512)
                fa = ffa.tile([128, CF, 512], BF16, tag="fa")
                for mf in range(CF):
                    fp = ps1.tile([128, 512], F32, tag="ps1")
                    for c in range(C):
                        nc.tensor.matmul(fp[:],
                                         f1_t[:, c, mf * 128:(mf + 1) * 128],
                                         xn2[:, c, sl],
                                         start=(c == 0), stop=(c == C - 1))
                    nc.scalar.activation(
                        fa[:, mf, :], fp[:], AF.Relu,
                        bias=fb1_t[:, i * CF + mf:i * CF + mf + 1])
                for mc in range(C):
                    f2p = ps1.tile([128, 512], F32, tag="ps1")
                    for c16 in range(CF):
                        nc.tensor.matmul(f2p[:],
                                         f2_t[:, c16, mc * 128:(mc + 1) * 128],
                                         fa[:, c16, :],
                                         start=(c16 == 0),
                                         stop=(c16 == CF - 1))
                    nc.vector.scalar_tensor_tensor(
                        out=h_t[:, mc, sl], in0=f2p[:],
                        scalar=fb2_t[:, i * C + mc:i * C + mc + 1],
                        in1=h_t[:, mc, sl], op0=OP.add, op1=OP.add)
                # peel next layer's LN1(tt) here so its scalar/vector chain
                # hides behind the other token-tile's FFN matmuls
                if i + 1 < nb_run:
                    if tt == 0:
                        ln_next = _alloc_ln(pools)
                    g_ap, b_ap, triv = ln_params(2 * (i + 1))
                    _emit_ln_tt(nc, pools, h_t, ones_t, eps2_t, g_ap, b_ap,
                                triv, tt, *ln_next)
                    if tt == 1:
                        xn_next = ln_next[2]

        # ---- final LN + logits ----
        g_ap, b_ap, triv = (ln_params(2 * NB) if nb_run == NB
                            else (None, None, True))
        xnf = _emit_ln(nc, pools, h_t, ones_t, eps2_t, g_ap, b_ap, triv)
        lg_sb = consts.tile([V, NTOK], F32)
        for tt in range(2):
            sl = slice(tt * 512, tt * 512 + 512)
            lg = ps1.tile([V, 512], F32, tag="ps1")
            for c in range(C):
                nc.tensor.matmul(lg[:], ow_t[:, c, :], xnf[:, c, sl],
                                 start=(c == 0), stop=(c == C - 1))
            nc.vector.tensor_scalar_add(lg_sb[:, sl], lg[:], ob_t[:])
        nc.sync.dma_start(out_d[:], lg_sb[:])

    nc.finalize()
    return nc


def prepare_inputs(inputs):
    """Host-side preprocessing: embedding gather, weight layout + bf16 cast.
    Returns (shared_map, per_core_h0_list, ln_trivial)."""
    f32 = np.float32
    bf16 = ml_dtypes.bfloat16
    x = np.asarray(inputs["x"]).astype(np.int64)
    emb = np.asarray(inputs["emb"], dtype=f32)
    pos = np.asarray(inputs["pos"], dtype=f32)

    positions = np.minimum(np.arange(T), L - 1)
    h0 = emb[x] + pos[positions][None, :, :]      # [B, T, E] fp32

    def to_dev_lhst(mat, kchunks, mcols):
        m = np.ascontiguousarray(mat.astype(bf16))
        return m.reshape(kchunks, 128, mcols).transpose(1, 0, 2).reshape(
            128, kchunks * mcols)

    wq = np.asarray(inputs["wq"], dtype=f32)
    wk = np.asarray(inputs["wk"], dtype=f32)
    wv = np.asarray(inputs["wv"], dtype=f32)
    pw = np.asarray(inputs["proj_w"], dtype=f32)
    f1 = np.asarray(inputs["ff_w1"], dtype=f32)
    f2 = np.asarray(inputs["ff_w2"], dtype=f32)

    wq_dev = np.stack([to_dev_lhst(wq[i].transpose(1, 0, 2).reshape(E, NH * HS),
                                   C, 512) for i in range(NB)])
    wk_dev = np.stack([to_dev_lhst(wk[i].transpose(1, 0, 2).reshape(E, NH * HS),
                                   C, 512) for i in range(NB)])
    wv_dev = np.stack([to_dev_lhst(wv[i].transpose(1, 0, 2).reshape(E, NH * HS),
                                   C, 512) for i in range(NB)])
    pw_dev = np.stack([to_dev_lhst(pw[i], C, 512) for i in range(NB)])
    f1_dev = np.stack([to_dev_lhst(f1[i], C, FF) for i in range(NB)])
    f2_dev = np.stack([to_dev_lhst(f2[i], CF, 512) for i in range(NB)])

    def vec_dev(v, chunks):
        return np.ascontiguousarray(v.astype(f32).reshape(chunks, 128).T)

    pb_dev = np.concatenate([vec_dev(np.asarray(inputs["proj_b"][i]), C)
                             for i in range(NB)], axis=1)
    fb1_dev = np.concatenate([vec_dev(np.asarray(inputs["ff_b1"][i]), CF)
                              for i in range(NB)], axis=1)
    fb2_dev = np.concatenate([vec_dev(np.asarray(inputs["ff_b2"][i]), C)
                              for i in range(NB)], axis=1)
    ow_dev = to_dev_lhst(np.asarray(inputs["out_w"], dtype=f32) / TEMP, C, V)
    ob_dev = (np.asarray(inputs["out_b"], dtype=f32) / TEMP).reshape(V, 1)
    tri_dev = np.triu(np.ones((128, 128), dtype=f32)).astype(bf16)

    gs, bs, ln_trivial = [], [], []
    for i in range(NB):
        for nm_g, nm_b in (("ln1_g", "ln1_b"), ("ln2_g", "ln2_b")):
            g = np.asarray(inputs[nm_g][i], dtype=f32)
            b = np.asarray(inputs[nm_b][i], dtype=f32)
            gs.append(vec_dev(g, C))
            bs.append(vec_dev(b, C))
            ln_trivial.append(bool(np.all(g == 1.0) and np.all(b == 0.0)))
    g = np.asarray(inputs["lnf_g"], dtype=f32)
    b = np.asarray(inputs["lnf_b"], dtype=f32)
    gs.append(vec_dev(g, C))
    bs.append(vec_dev(b, C))
    ln_trivial.append(bool(np.all(g == 1.0) and np.all(b == 0.0)))
    lng_dev = np.concatenate(gs, axis=1)
    lnb_dev = np.concatenate(bs, axis=1)

    shared = {
        "wq": wq_dev, "wk": wk_dev, "wv": wv_dev, "pw": pw_dev,
        "f1": f1_dev, "f2": f2_dev, "pb": pb_dev, "fb1": fb1_dev,
        "fb2": fb2_dev, "ow": ow_dev, "ob": ob_dev, "tri": tri_dev,
        "lng": lng_dev, "lnb": lnb_dev,
    }

    h0_cores = []
    for core in range(NCORES):
        hh = h0[SEQ * core:SEQ * core + SEQ]          # [SEQ, T, E]
        hT = hh.transpose(2, 0, 1).reshape(E, NTOK)   # [E, NTOK]
        h0_cores.append(np.ascontiguousarray(
            hT.reshape(C, 128, NTOK).transpose(1, 0, 2).reshape(
                128, C * NTOK)))
    return shared, h0_cores, ln_trivial


def assemble_output(core_logits):
    """core_logits: list of [V, NTOK] fp32 -> [B, T, V]."""
    out = np.empty((B, T, V), np.float32)
    for core in range(NCORES):
        lg = core_logits[core].reshape(V, SEQ, T)
        out[SEQ * core:SEQ * core + SEQ] = lg.transpose(1, 2, 0)
    return out


def get_program(ln_trivial):
    key = tuple(ln_trivial)
    if key not in _PROGRAM_CACHE:
        _PROGRAM_CACHE[key] = build_program(list(key))
    return _PROGRAM_CACHE[key]


def reset_device():
    """Recover a wedged accelerator (axon session reset). Best-effort."""
    try:
        import ctypes
        import jax
        jax.devices()
        lib = ctypes.CDLL('/opt/axon/libaxon_pjrt.so')
        lib.axon_reset.restype = ctypes.c_int64
        lib.axon_reset()
    except Exception:
        pass


def kernel(**inputs):
    from concourse.bass_utils import run_bass_kernel_spmd
    shared, h0_cores, ln_trivial = prepare_inputs(inputs)
    nc = get_program(ln_trivial)
    in_maps = [dict(shared, h0=h0_cores[c]) for c in range(NCORES)]
    try:
        res = run_bass_kernel_spmd(nc, in_maps, core_ids=list(range(NCORES)))
    except Exception:
        # A previous (profiled) session can leave the device wedged; reset
        # the axon session and retry once.
        reset_device()
        res = run_bass_kernel_spmd(nc, in_maps, core_ids=list(range(NCORES)))
    return assemble_output([res.results[c]["logits"] for c in range(NCORES)])


# revision 27
# speedup vs baseline: 1.0159x; 1.0159x over previous
"""Trainium2 Bass kernel for nn_AutoregressiveArithmeticTransformer.

6-layer dense transformer: B=16, T=512, E=512, NH=8 heads x HS=64, FF=2048,
V=16, causal attention, pre-LN, learned abacus embedding, logits / 0.8.

Strategy: data-parallel over batch across 8 NeuronCores (2 sequences per
core, no collectives). Activations live feature-major in SBUF
([E-partitions, tokens]); weights are streamed per-layer in bf16; all
matmuls run in bf16 with fp32 PSUM accumulation; the residual stream stays
fp32. LayerNorm statistics are computed with ones-matmuls on the PE;
attention scores are computed transposed ([tk, tq]) so the softmax
denominator is also a ones-matmul; V is produced token-major directly so
no transposes are ever needed.

All ops are token-tile (512) granular so the two sequences per core form
independent dependency streams the Tile scheduler can interleave.
"""

import numpy as np
import ml_dtypes

import concourse.bacc as bacc
import concourse.tile as tile
from concourse import mybir

F32 = mybir.dt.float32
BF16 = mybir.dt.bfloat16
AF = mybir.ActivationFunctionType
OP = mybir.AluOpType

# Model constants (hardcoded per contest contract)
V, E, NH, HS, FF, NB, L = 16, 512, 8, 64, 2048, 6, 512
B, T = 16, 512
TEMP = 1.0 * 0.8
EPS = 1e-5
SCALE = HS ** -0.5  # 0.125

NCORES = 8
SEQ = 2              # sequences per core
NTOK = SEQ * T       # 1024 tokens per core
C = E // 128         # 4 E-chunks
CF = FF // 128       # 16 FF-chunks
HP = NH // 2         # 4 head-pairs
NJ = T // 128        # 4 tk chunks per sequence

_PROGRAM_CACHE = {}


def _emit_ln_tt(nc, pools, h_t, ones_t, eps2_t, g_ap, b_ap, trivial, tt,
                hb, sq, xn):
    """One token-tile of LayerNorm into caller-allocated hb/sq/xn tiles."""
    stats, stats_bf = pools["stats"], pools["stats_bf"]
    ps1 = pools["ps1"]
    sl = slice(tt * 512, tt * 512 + 512)
    s1 = ps1.tile([128, 512], F32, tag="ps1")
    s2 = ps1.tile([128, 512], F32, tag="ps1")
    for c in range(C):
        nc.scalar.copy(hb[:, c, sl], h_t[:, c, sl])
        nc.vector.tensor_tensor(sq[:, c, sl], hb[:, c, sl], hb[:, c, sl],
                                OP.mult)
        nc.tensor.matmul(s1[:], ones_t[:], hb[:, c, sl],
                         start=(c == 0), stop=(c == C - 1))
        nc.tensor.matmul(s2[:], ones_t[:], sq[:, c, sl],
                         start=(c == 0), stop=(c == C - 1))
    msq = stats.tile([128, 512], F32, tag="stats")
    nc.scalar.square(msq[:], s1[:])
    var = stats.tile([128, 512], F32, tag="stats")
    nc.vector.scalar_tensor_tensor(out=var[:], in0=s2[:], scalar=float(E),
                                   in1=msq[:], op0=OP.mult, op1=OP.subtract)
    std = stats.tile([128, 512], F32, tag="stats")
    nc.scalar.activation(std[:], var[:], AF.Sqrt, bias=eps2_t[:])
    rc = stats.tile([128, 512], F32, tag="stats")
    nc.vector.reciprocal_approx_fast(out=rc[:], in_=std[:])
    r_bf = stats_bf.tile([128, 512], BF16, tag="r_bf")
    nc.scalar.mul(r_bf[:], rc[:], float(E))
    z_bf = stats_bf.tile([128, 512], BF16, tag="z_bf")
    nc.vector.tensor_tensor(z_bf[:], s1[:], rc[:], OP.mult)
    for c in range(C):
        nc.vector.tensor_tensor(xn[:, c, sl], hb[:, c, sl], r_bf[:], OP.mult)
        nc.vector.tensor_tensor(xn[:, c, sl], xn[:, c, sl], z_bf[:],
                                OP.subtract)
        if not trivial:
            nc.vector.tensor_scalar(out=xn[:, c, sl], in0=xn[:, c, sl],
                                    scalar1=g_ap[:, c:c + 1],
                                    scalar2=b_ap[:, c:c + 1],
                                    op0=OP.mult, op1=OP.add)


def _alloc_ln(pools):
    hb = pools["scr"].tile([128, C, NTOK], BF16, tag="scratch", name="hb")
    sq = pools["scr2"].tile([128, C, NTOK], BF16, tag="sq", name="sq")
    xn = pools["scr"].tile([128, C, NTOK], BF16, tag="scratch", name="xnt")
    return hb, sq, xn


def _emit_ln(nc, pools, h_t, ones_t, eps2_t, g_ap, b_ap, trivial):
    hb, sq, xn = _alloc_ln(pools)
    for tt in range(2):
        _emit_ln_tt(nc, pools, h_t, ones_t, eps2_t, g_ap, b_ap, trivial, tt,
                    hb, sq, xn)
    return xn


def build_program(ln_trivial, nb_run=NB, ln_general_params=True):
    """Build the Bass program. ln_trivial: list of NB*2+1 bools (ln1/ln2 per
    layer then lnf) -- when True the g/b application op is skipped."""
    nc = bacc.Bacc(None, target_bir_lowering=False)

    h0_d = nc.dram_tensor("h0", [128, C * NTOK], F32, kind="ExternalInput")
    wq_d = nc.dram_tensor("wq", [NB, 128, C * 512], BF16, kind="ExternalInput")
    wk_d = nc.dram_tensor("wk", [NB, 128, C * 512], BF16, kind="ExternalInput")
    wv_d = nc.dram_tensor("wv", [NB, 128, C * 512], BF16, kind="ExternalInput")
    pw_d = nc.dram_tensor("pw", [NB, 128, C * 512], BF16, kind="ExternalInput")
    f1_d = nc.dram_tensor("f1", [NB, 128, C * FF], BF16, kind="ExternalInput")
    f2_d = nc.dram_tensor("f2", [NB, 128, CF * 512], BF16, kind="ExternalInput")
    pb_d = nc.dram_tensor("pb", [128, NB * C], F32, kind="ExternalInput")
    fb1_d = nc.dram_tensor("fb1", [128, NB * CF], F32, kind="ExternalInput")
    fb2_d = nc.dram_tensor("fb2", [128, NB * C], F32, kind="ExternalInput")
    ow_d = nc.dram_tensor("ow", [128, C * V], BF16, kind="ExternalInput")
    ob_d = nc.dram_tensor("ob", [V, 1], F32, kind="ExternalInput")
    tri_d = nc.dram_tensor("tri", [128, 128], BF16, kind="ExternalInput")
    lng_d = lnb_d = None
    if ln_general_params:
        lng_d = nc.dram_tensor("lng", [128, (2 * NB + 1) * C], F32,
                               kind="ExternalInput")
        lnb_d = nc.dram_tensor("lnb", [128, (2 * NB + 1) * C], F32,
                               kind="ExternalInput")
    out_d = nc.dram_tensor("logits", [V, NTOK], F32, kind="ExternalOutput")

    from contextlib import ExitStack
    with ExitStack() as ctx:
        tc = ctx.enter_context(tile.TileContext(nc))
        consts = ctx.enter_context(tc.tile_pool(name="consts", bufs=1))
        hpool = ctx.enter_context(tc.tile_pool(name="hpool", bufs=1))
        wqkv = ctx.enter_context(tc.tile_pool(name="wqkv", bufs=1))
        wff1 = ctx.enter_context(tc.tile_pool(name="wff1", bufs=1))
        wff2 = ctx.enter_context(tc.tile_pool(name="wff2", bufs=1))
        scr = ctx.enter_context(tc.tile_pool(name="scr", bufs=4))
        scr2 = ctx.enter_context(tc.tile_pool(name="scr2", bufs=1))
        qk = ctx.enter_context(tc.tile_pool(name="qk", bufs=2))
        vt = ctx.enter_context(tc.tile_pool(name="vt", bufs=1))
        pp = ctx.enter_context(tc.tile_pool(name="pp", bufs=3))
        osb = ctx.enter_context(tc.tile_pool(name="osb", bufs=1))
        ffa = ctx.enter_context(tc.tile_pool(name="ffa", bufs=2))
        stats = ctx.enter_context(tc.tile_pool(name="stats", bufs=6))
        stats_bf = ctx.enter_context(tc.tile_pool(name="stats_bf", bufs=2))
        ps1 = ctx.enter_context(tc.tile_pool(name="ps1", bufs=8, space="PSUM"))
        ps2 = ps1

        pools = {"scr": scr, "scr2": scr2, "stats": stats,
                 "stats_bf": stats_bf, "ps2": ps2, "ps1": ps1}

        ones_t = consts.tile([128, 128], BF16)
        nc.gpsimd.memset(ones_t[:], 1.0)
        eps2_t = consts.tile([128, 1], F32)
        nc.gpsimd.memset(eps2_t[:], float(E) * float(E) * EPS)
        tri_t = consts.tile([128, 128], BF16)
        nc.sync.dma_start(tri_t[:], tri_d[:])
        pb_t = consts.tile([128, NB * C], F32)
        nc.sync.dma_start(pb_t[:], pb_d[:])
        fb1_t = consts.tile([128, NB * CF], F32)
        nc.sync.dma_start(fb1_t[:], fb1_d[:])
        fb2_t = consts.tile([128, NB * C], F32)
        nc.sync.dma_start(fb2_t[:], fb2_d[:])
        ow_t = consts.tile([128, C, V], BF16)
        nc.sync.dma_start(ow_t[:], ow_d[:].rearrange("p (c v) -> p c v", v=V))
        ob_t = consts.tile([V, 1], F32)
        nc.sync.dma_start(ob_t[:], ob_d[:])
        lng_t = lnb_t = None
        if ln_general_params:
            lng_t = consts.tile([128, 2 * NB + 1, C], F32)
            nc.sync.dma_start(lng_t[:], lng_d[:].rearrange(
                "p (l c) -> p l c", c=C))
            lnb_t = consts.tile([128, 2 * NB + 1, C], F32)
            nc.sync.dma_start(lnb_t[:], lnb_d[:].rearrange(
                "p (l c) -> p l c", c=C))

        h_t = hpool.tile([128, C, NTOK], F32)
        nc.sync.dma_start(h_t[:], h0_d[:].rearrange(
            "p (c t) -> p c t", t=NTOK))

        def ln_params(idx):
            if ln_general_params and not ln_trivial[idx]:
                return lng_t[:, idx, :], lnb_t[:, idx, :], False
            return None, None, True

        for i in range(nb_run):
            # ---- load this layer's weights ----
            wq_t = wqkv.tile([128, C, 512], BF16, tag="wq")
            nc.sync.dma_start(wq_t[:], wq_d[i].rearrange(
                "p (c m) -> p c m", m=512))
            wk_t = wqkv.tile([128, C, 512], BF16, tag="wk")
            nc.sync.dma_start(wk_t[:], wk_d[i].rearrange(
                "p (c m) -> p c m", m=512))
            wv_t = wqkv.tile([128, C, 512], BF16, tag="wv")
            nc.sync.dma_start(wv_t[:], wv_d[i].rearrange(
                "p (c m) -> p c m", m=512))
            pw_t = wqkv.tile([128, C, 512], BF16, tag="pw")
            nc.sync.dma_start(pw_t[:], pw_d[i].rearrange(
                "p (c m) -> p c m", m=512))
            f1_t = wff1.tile([128, C, FF], BF16, tag="f1")
            nc.sync.dma_start(f1_t[:], f1_d[i].rearrange(
                "p (c m) -> p c m", m=FF))
            f2_t = wff2.tile([128, CF, 512], BF16, tag="f2")
            nc.sync.dma_start(f2_t[:], f2_d[i].rearrange(
                "p (c m) -> p c m", m=512))

            # ---- LN1 (layer 0: pre-peeled below; others peeled into
            #      the previous layer's FFN emission) ----
            if i == 0:
                g_ap, b_ap, triv = ln_params(0)
                xn = _emit_ln(nc, pools, h_t, ones_t, eps2_t, g_ap, b_ap,
                              triv)
            else:
                xn = xn_next

            # ---- V projection, token-major: vT[tk, hd*64+d] ----
            vt_t = vt.tile([128, SEQ * NJ, 512], BF16, tag="vt")
            for jg in range(SEQ * NJ):
                vp = ps1.tile([128, 512], F32, tag="ps1")
                for c in range(C):
                    nc.tensor.matmul(vp[:], xn[:, c, jg * 128:(jg + 1) * 128],
                                     wv_t[:, c, :],
                                     start=(c == 0), stop=(c == C - 1))
                nc.scalar.copy(vt_t[:, jg, :], vp[:])

            o_t = osb.tile([128, C, NTOK], BF16, tag="o")

            def emit_den_o(s, hp, p_t):
                base = s * T
                rdens = []
                for h2 in range(2):
                    den = ps1.tile([128, 512], F32, tag="ps1")
                    for j in range(NJ):
                        off = j * 128
                        njw = T - off
                        nc.tensor.matmul(den[:, off:512], ones_t[:],
                                         p_t[:, h2, j, 0:njw],
                                         start=(j == 0), stop=(j == NJ - 1))
                    rd = stats.tile([128, 512], F32, tag="stats")
                    nc.vector.reciprocal_approx_fast(out=rd[:], in_=den[:])
                    rdens.append(rd)
                op_ps = ps1.tile([128, 512], F32, tag="ps1")
                for h2 in range(2):
                    head = hp * 2 + h2
                    for j in range(NJ):
                        off = j * 128
                        njw = T - off
                        nc.tensor.matmul(
                            op_ps[h2 * 64:h2 * 64 + 64, off:T],
                            vt_t[:, s * NJ + j, head * 64:head * 64 + 64],
                            p_t[:, h2, j, 0:njw],
                            start=(j == 0), stop=(j == NJ - 1))
                for h2 in range(2):
                    dsl = slice(h2 * 64, h2 * 64 + 64)
                    nc.vector.tensor_tensor(
                        o_t[dsl, hp, base:base + T], op_ps[dsl, 0:T],
                        rdens[h2][dsl, :], OP.mult)

            pending = None
            for hp in range(HP):
                msl = slice(hp * 128, (hp + 1) * 128)
                q_t = qk.tile([128, NTOK], BF16, tag="q")
                k_t = qk.tile([128, NTOK], BF16, tag="k")
                for tt in range(2):
                    sl = slice(tt * 512, tt * 512 + 512)
                    qp = ps1.tile([128, 512], F32, tag="ps1")
                    kp = ps1.tile([128, 512], F32, tag="ps1")
                    for c in range(C):
                        nc.tensor.matmul(qp[:], wq_t[:, c, msl],
                                         xn[:, c, sl],
                                         start=(c == 0), stop=(c == C - 1))
                        nc.tensor.matmul(kp[:], wk_t[:, c, msl],
                                         xn[:, c, sl],
                                         start=(c == 0), stop=(c == C - 1))
                    nc.scalar.copy(q_t[:, sl], qp[:])
                    nc.scalar.copy(k_t[:, sl], kp[:])

                for s in range(SEQ):
                    base = s * T
                    p_t = pp.tile([128, 2, NJ, 512], BF16, tag="p")
                    for j in range(NJ):
                        off = j * 128
                        njw = T - off
                        for h2 in range(2):
                            dsl = slice(h2 * 64, h2 * 64 + 64)
                            sT = ps1.tile([128, 512], F32, tag="ps1")
                            nc.tensor.matmul(
                                sT[:, 0:njw],
                                k_t[dsl, base + off:base + off + 128],
                                q_t[dsl, base + off:base + T],
                                start=True, stop=True)
                            nc.scalar.activation(
                                p_t[:, h2, j, 0:njw], sT[:, 0:njw],
                                AF.Exp, scale=SCALE)
                        nc.vector.tensor_tensor(
                            p_t[:, :, j, 0:128], p_t[:, :, j, 0:128],
                            tri_t[:, None, :].to_broadcast(
                                (128, 2, 128)), OP.mult)
                    if pending is not None:
                        emit_den_o(*pending)
                    pending = (s, hp, p_t)
            emit_den_o(*pending)

            # ---- attention out projection + residual ----
            for tt in range(2):
                sl = slice(tt * 512, tt * 512 + 512)
                for mc in range(C):
                    pj = ps1.tile([128, 512], F32, tag="ps1")
                    for c in range(C):
                        nc.tensor.matmul(pj[:],
                                         pw_t[:, c, mc * 128:(mc + 1) * 128],
                                         o_t[:, c, sl],
                                         start=(c == 0), stop=(c == C - 1))
                    nc.vector.scalar_tensor_tensor(
                        out=h_t[:, mc, sl], in0=pj[:],
                        scalar=pb_t[:, i * C + mc:i * C + mc + 1],
                        in1=h_t[:, mc, sl], op0=OP.add, op1=OP.add)

            # ---- LN2 + FFN (token-tile split) ----
            g_ap, b_ap, triv = ln_params(2 * i + 1)
            xn2 = _emit_ln(nc, pools, h_t, ones_t, eps2_t, g_ap, b_ap, triv)

            for tt in range(2):
                sl = slice(tt * 512, tt * 512 + 512)
                fa = ffa.tile([128, CF, 512], BF16, tag="fa")
                for mf in range(CF):
                    fp = ps1.tile([128, 512], F32, tag="ps1")
                    for c in range(C):
                        nc.tensor.matmul(fp[:],
                                         f1_t[:, c, mf * 128:(mf + 1) * 128],
                                         xn2[:, c, sl],
                                         start=(c == 0), stop=(c == C - 1))
                    nc.scalar.activation(
                        fa[:, mf, :], fp[:], AF.Relu,
                        bias=fb1_t[:, i * CF + mf:i * CF + mf + 1])
                for mc in range(C):
                    f2p = ps1.tile([128, 512], F32, tag="ps1")
                    for c16 in range(CF):
                        nc.tensor.matmul(f2p[:],
                                         f2_t[:, c16, mc * 128:(mc + 1) * 128],
                                         fa[:, c16, :],
                                         start=(c16 == 0),
                                         stop=(c16 == CF - 1))
                    nc.vector.scalar_tensor_tensor(
                        out=h_t[:, mc, sl], in0=f2p[:],
                        scalar=fb2_t[:, i * C + mc:i * C + mc + 1],
                        in1=h_t[:, mc, sl], op0=OP.add, op1=OP.add)
                # peel next layer's LN1(tt) here so its scalar/vector chain
                # hides behind the other token-tile's FFN matmuls
                if i + 1 < nb_run:
                    if tt == 0:
                        ln_next = _alloc_ln(pools)
                    g_ap, b_ap, triv = ln_params(2 * (i + 1))
                    _emit_ln_tt(nc, pools, h_t, ones_t, eps2_t, g_ap, b_ap,
                                triv, tt, *ln_next)
                    if tt == 1:
                        xn_next = ln_next[2]

        # ---- final LN + logits ----
        g_ap, b_ap, triv = (ln_params(2 * NB) if nb_run == NB
                            else (None, None, True))
        xnf = _emit_ln(nc, pools, h_t, ones_t, eps2_t, g_ap, b_ap, triv)
        lg_sb = consts.tile([V, NTOK], F32)
        for tt in range(2):
            sl = slice(tt * 512, tt * 512 + 512)
            lg = ps1.tile([V, 512], F32, tag="ps1")
            for c in range(C):
                nc.tensor.matmul(lg[:], ow_t[:, c, :], xnf[:, c, sl],
                                 start=(c == 0), stop=(c == C - 1))
            nc.vector.tensor_scalar_add(lg_sb[:, sl], lg[:], ob_t[:])
        nc.sync.dma_start(out_d[:], lg_sb[:])

    nc.finalize()
    return nc


def prepare_inputs(inputs):
    """Host-side preprocessing: embedding gather, weight layout + bf16 cast.
    Returns (shared_map, per_core_h0_list, ln_trivial)."""
    f32 = np.float32
    bf16 = ml_dtypes.bfloat16
    x = np.asarray(inputs["x"]).astype(np.int64)
    emb = np.asarray(inputs["emb"], dtype=f32)
    pos = np.asarray(inputs["pos"], dtype=f32)

    positions = np.minimum(np.arange(T), L - 1)
    h0 = emb[x] + pos[positions][None, :, :]      # [B, T, E] fp32

    def to_dev_lhst(mat, kchunks, mcols):
        m = np.ascontiguousarray(mat.astype(bf16))
        return m.reshape(kchunks, 128, mcols).transpose(1, 0, 2).reshape(
            128, kchunks * mcols)

    wq = np.asarray(inputs["wq"], dtype=f32)
    wk = np.asarray(inputs["wk"], dtype=f32)
    wv = np.asarray(inputs["wv"], dtype=f32)
    pw = np.asarray(inputs["proj_w"], dtype=f32)
    f1 = np.asarray(inputs["ff_w1"], dtype=f32)
    f2 = np.asarray(inputs["ff_w2"], dtype=f32)

    wq_dev = np.stack([to_dev_lhst(wq[i].transpose(1, 0, 2).reshape(E, NH * HS),
                                   C, 512) for i in range(NB)])
    wk_dev = np.stack([to_dev_lhst(wk[i].transpose(1, 0, 2).reshape(E, NH * HS),
                                   C, 512) for i in range(NB)])
    wv_dev = np.stack([to_dev_lhst(wv[i].transpose(1, 0, 2).reshape(E, NH * HS),
                                   C, 512) for i in range(NB)])
    pw_dev = np.stack([to_dev_lhst(pw[i], C, 512) for i in range(NB)])
    f1_dev = np.stack([to_dev_lhst(f1[i], C, FF) for i in range(NB)])
    f2_dev = np.stack([to_dev_lhst(f2[i], CF, 512) for i in range(NB)])

    def vec_dev(v, chunks):
        return np.ascontiguousarray(v.astype(f32).reshape(chunks, 128).T)

    pb_dev = np.concatenate([vec_dev(np.asarray(inputs["proj_b"][i]), C)
                             for i in range(NB)], axis=1)
    fb1_dev = np.concatenate([vec_dev(np.asarray(inputs["ff_b1"][i]), CF)
                              for i in range(NB)], axis=1)
    fb2_dev = np.concatenate([vec_dev(np.asarray(inputs["ff_b2"][i]), C)
                              for i in range(NB)], axis=1)
    ow_dev = to_dev_lhst(np.asarray(inputs["out_w"], dtype=f32) / TEMP, C, V)
    ob_dev = (np.asarray(inputs["out_b"], dtype=f32) / TEMP).reshape(V, 1)
    tri_dev = np.triu(np.ones((128, 128), dtype=f32)).astype(bf16)

    gs, bs, ln_trivial = [], [], []
    for i in range(NB):
        for nm_g, nm_b in (("ln1_g", "ln1_b"), ("ln2_g", "ln2_b")):
            g = np.asarray(inputs[nm_g][i], dtype=f32)
            b = np.asarray(inputs[nm_b][i], dtype=f32)
            gs.append(vec_dev(g, C))
            bs.append(vec_dev(b, C))
            ln_trivial.append(bool(np.all(g == 1.0) and np.all(b == 0.0)))
    g = np.asarray(inputs["lnf_g"], dtype=f32)
    b = np.asarray(inputs["lnf_b"], dtype=f32)
    gs.append(vec_dev(g, C))
    bs.append(vec_dev(b, C))
    ln_trivial.append(bool(np.all(g == 1.0) and np.all(b == 0.0)))
    lng_dev = np.concatenate(gs, axis=1)
    lnb_dev = np.concatenate(bs, axis=1)

    shared = {
        "wq": wq_dev, "wk": wk_dev, "wv": wv_dev, "pw": pw_dev,
        "f1": f1_dev, "f2": f2_dev, "pb": pb_dev, "fb1": fb1_dev,
        "fb2": fb2_dev, "ow": ow_dev, "ob": ob_dev, "tri": tri_dev,
        "lng": lng_dev, "lnb": lnb_dev,
    }

    h0_cores = []
    for core in range(NCORES):
        hh = h0[SEQ * core:SEQ * core + SEQ]          # [SEQ, T, E]
        hT = hh.transpose(2, 0, 1).reshape(E, NTOK)   # [E, NTOK]
        h0_cores.append(np.ascontiguousarray(
            hT.reshape(C, 128, NTOK).transpose(1, 0, 2).reshape(
                128, C * NTOK)))
    return shared, h0_cores, ln_trivial


def assemble_output(core_logits):
    """core_logits: list of [V, NTOK] fp32 -> [B, T, V]."""
    out = np.empty((B, T, V), np.float32)
    for core in range(NCORES):
        lg = core_logits[core].reshape(V, SEQ, T)
        out[SEQ * core:SEQ * core + SEQ] = lg.transpose(1, 2, 0)
    return out


def get_program(ln_trivial):
    key = tuple(ln_trivial)
    if key not in _PROGRAM_CACHE:
        _PROGRAM_CACHE[key] = build_program(list(key))
    return _PROGRAM_CACHE[key]


def reset_device():
    """Recover a wedged accelerator (axon session reset). Best-effort."""
    try:
        import ctypes
        import jax
        jax.devices()
        lib = ctypes.CDLL('/opt/axon/libaxon_pjrt.so')
        lib.axon_reset.restype = ctypes.c_int64
        lib.axon_reset()
    except Exception:
        pass


def kernel(**inputs):
    from concourse.bass_utils import run_bass_kernel_spmd
    shared, h0_cores, ln_trivial = prepare_inputs(inputs)
    nc = get_program(ln_trivial)
    in_maps = [dict(shared, h0=h0_cores[c]) for c in range(NCORES)]
    try:
        res = run_bass_kernel_spmd(nc, in_maps, core_ids=list(range(NCORES)))
    except Exception:
        # A previous (profiled) session can leave the device wedged; reset
        # the axon session and retry once.
        reset_device()
        res = run_bass_kernel_spmd(nc, in_maps, core_ids=list(range(NCORES)))
    return assemble_output([res.results[c]["logits"] for c in range(NCORES)])


# revision 28
# speedup vs baseline: 1.0527x; 1.0362x over previous
"""Trainium2 Bass kernel for nn_AutoregressiveArithmeticTransformer.

6-layer dense transformer: B=16, T=512, E=512, NH=8 heads x HS=64, FF=2048,
V=16, causal attention, pre-LN, learned abacus embedding, logits / 0.8.

Strategy: data-parallel over batch across 8 NeuronCores (2 sequences per
core, no collectives). Activations live feature-major in SBUF
([E-partitions, tokens]); weights are streamed per-layer in bf16; all
matmuls run in bf16 with fp32 PSUM accumulation; the residual stream stays
fp32. LayerNorm statistics are computed with ones-matmuls on the PE;
attention scores are computed transposed ([tk, tq]) so the softmax
denominator is also a ones-matmul; V is produced token-major directly so
no transposes are ever needed.

All ops are token-tile (512) granular so the two sequences per core form
independent dependency streams the Tile scheduler can interleave.
"""

import numpy as np
import ml_dtypes

import concourse.bacc as bacc
import concourse.tile as tile
from concourse import mybir

F32 = mybir.dt.float32
BF16 = mybir.dt.bfloat16
AF = mybir.ActivationFunctionType
OP = mybir.AluOpType

# Model constants (hardcoded per contest contract)
V, E, NH, HS, FF, NB, L = 16, 512, 8, 64, 2048, 6, 512
B, T = 16, 512
TEMP = 1.0 * 0.8
EPS = 1e-5
SCALE = HS ** -0.5  # 0.125

NCORES = 8
SEQ = 2              # sequences per core
NTOK = SEQ * T       # 1024 tokens per core
C = E // 128         # 4 E-chunks
CF = FF // 128       # 16 FF-chunks
HP = NH // 2         # 4 head-pairs
NJ = T // 128        # 4 tk chunks per sequence

_PROGRAM_CACHE = {}


def _emit_ln_tt(nc, pools, h_t, ones_t, eps2_t, g_ap, b_ap, trivial, tt,
                hb, sq, xn):
    """One token-tile of LayerNorm into caller-allocated hb/sq/xn tiles."""
    stats, stats_bf = pools["stats"], pools["stats_bf"]
    ps1 = pools["ps1"]
    sl = slice(tt * 512, tt * 512 + 512)
    s1 = ps1.tile([128, 512], F32, tag="ps1")
    s2 = ps1.tile([128, 512], F32, tag="ps1")
    for c in range(C):
        nc.scalar.copy(hb[:, c, sl], h_t[:, c, sl])
        nc.vector.tensor_tensor(sq[:, c, sl], hb[:, c, sl], hb[:, c, sl],
                                OP.mult)
        nc.tensor.matmul(s1[:], ones_t[:], hb[:, c, sl],
                         start=(c == 0), stop=(c == C - 1))
        nc.tensor.matmul(s2[:], ones_t[:], sq[:, c, sl],
                         start=(c == 0), stop=(c == C - 1))
    msq = stats.tile([128, 512], F32, tag="stats")
    nc.scalar.square(msq[:], s1[:])
    var = stats.tile([128, 512], F32, tag="stats")
    nc.vector.scalar_tensor_tensor(out=var[:], in0=s2[:], scalar=float(E),
                                   in1=msq[:], op0=OP.mult, op1=OP.subtract)
    std = stats.tile([128, 512], F32, tag="stats")
    nc.scalar.activation(std[:], var[:], AF.Sqrt, bias=eps2_t[:])
    rc = stats.tile([128, 512], F32, tag="stats")
    nc.vector.reciprocal_approx_fast(out=rc[:], in_=std[:])
    r_bf = stats_bf.tile([128, 512], BF16, tag="r_bf")
    nc.scalar.mul(r_bf[:], rc[:], float(E))
    z_bf = stats_bf.tile([128, 512], BF16, tag="z_bf")
    nc.vector.tensor_tensor(z_bf[:], s1[:], rc[:], OP.mult)
    for c in range(C):
        nc.vector.tensor_tensor(xn[:, c, sl], hb[:, c, sl], r_bf[:], OP.mult)
        nc.vector.tensor_tensor(xn[:, c, sl], xn[:, c, sl], z_bf[:],
                                OP.subtract)
        if not trivial:
            nc.vector.tensor_scalar(out=xn[:, c, sl], in0=xn[:, c, sl],
                                    scalar1=g_ap[:, c:c + 1],
                                    scalar2=b_ap[:, c:c + 1],
                                    op0=OP.mult, op1=OP.add)


def _alloc_ln(pools):
    hb = pools["scr"].tile([128, C, NTOK], BF16, tag="scratch", name="hb")
    sq = pools["scr2"].tile([128, C, NTOK], BF16, tag="sq", name="sq")
    xn = pools["scr"].tile([128, C, NTOK], BF16, tag="scratch", name="xnt")
    return hb, sq, xn


def _emit_ln(nc, pools, h_t, ones_t, eps2_t, g_ap, b_ap, trivial):
    hb, sq, xn = _alloc_ln(pools)
    for tt in range(2):
        _emit_ln_tt(nc, pools, h_t, ones_t, eps2_t, g_ap, b_ap, trivial, tt,
                    hb, sq, xn)
    return xn


def build_program(ln_trivial, nb_run=NB, ln_general_params=True):
    """Build the Bass program. ln_trivial: list of NB*2+1 bools (ln1/ln2 per
    layer then lnf) -- when True the g/b application op is skipped."""
    nc = bacc.Bacc(None, target_bir_lowering=False)

    h0_d = nc.dram_tensor("h0", [128, C * NTOK], F32, kind="ExternalInput")
    wq_d = nc.dram_tensor("wq", [NB, 128, C * 512], BF16, kind="ExternalInput")
    wk_d = nc.dram_tensor("wk", [NB, 128, C * 512], BF16, kind="ExternalInput")
    wv_d = nc.dram_tensor("wv", [NB, 128, C * 512], BF16, kind="ExternalInput")
    pw_d = nc.dram_tensor("pw", [NB, 128, C * 512], BF16, kind="ExternalInput")
    f1_d = nc.dram_tensor("f1", [NB, 128, C * FF], BF16, kind="ExternalInput")
    f2_d = nc.dram_tensor("f2", [NB, 128, CF * 512], BF16, kind="ExternalInput")
    pb_d = nc.dram_tensor("pb", [128, NB * C], F32, kind="ExternalInput")
    fb1_d = nc.dram_tensor("fb1", [128, NB * CF], F32, kind="ExternalInput")
    fb2_d = nc.dram_tensor("fb2", [128, NB * C], F32, kind="ExternalInput")
    ow_d = nc.dram_tensor("ow", [128, C * V], BF16, kind="ExternalInput")
    ob_d = nc.dram_tensor("ob", [V, 1], F32, kind="ExternalInput")
    tri_d = nc.dram_tensor("tri", [128, 128], BF16, kind="ExternalInput")
    lng_d = lnb_d = None
    if ln_general_params:
        lng_d = nc.dram_tensor("lng", [128, (2 * NB + 1) * C], F32,
                               kind="ExternalInput")
        lnb_d = nc.dram_tensor("lnb", [128, (2 * NB + 1) * C], F32,
                               kind="ExternalInput")
    out_d = nc.dram_tensor("logits", [V, NTOK], F32, kind="ExternalOutput")

    from contextlib import ExitStack
    with ExitStack() as ctx:
        tc = ctx.enter_context(tile.TileContext(nc))
        consts = ctx.enter_context(tc.tile_pool(name="consts", bufs=1))
        hpool = ctx.enter_context(tc.tile_pool(name="hpool", bufs=1))
        wqkv = ctx.enter_context(tc.tile_pool(name="wqkv", bufs=1))
        wff1 = ctx.enter_context(tc.tile_pool(name="wff1", bufs=1))
        wff2 = ctx.enter_context(tc.tile_pool(name="wff2", bufs=1))
        scr = ctx.enter_context(tc.tile_pool(name="scr", bufs=4))
        scr2 = ctx.enter_context(tc.tile_pool(name="scr2", bufs=1))
        qk = ctx.enter_context(tc.tile_pool(name="qk", bufs=2))
        vt = ctx.enter_context(tc.tile_pool(name="vt", bufs=1))
        pp = ctx.enter_context(tc.tile_pool(name="pp", bufs=3))
        osb = ctx.enter_context(tc.tile_pool(name="osb", bufs=1))
        ffa = ctx.enter_context(tc.tile_pool(name="ffa", bufs=2))
        stats = ctx.enter_context(tc.tile_pool(name="stats", bufs=6))
        stats_bf = ctx.enter_context(tc.tile_pool(name="stats_bf", bufs=2))
        ps1 = ctx.enter_context(tc.tile_pool(name="ps1", bufs=8, space="PSUM"))
        ps2 = ps1

        pools = {"scr": scr, "scr2": scr2, "stats": stats,
                 "stats_bf": stats_bf, "ps2": ps2, "ps1": ps1}

        ones_t = consts.tile([128, 128], BF16)
        nc.gpsimd.memset(ones_t[:], 1.0)
        eps2_t = consts.tile([128, 1], F32)
        nc.gpsimd.memset(eps2_t[:], float(E) * float(E) * EPS)
        tri_t = consts.tile([128, 128], BF16)
        nc.sync.dma_start(tri_t[:], tri_d[:])
        pb_t = consts.tile([128, NB * C], F32)
        nc.sync.dma_start(pb_t[:], pb_d[:])
        fb1_t = consts.tile([128, NB * CF], F32)
        nc.sync.dma_start(fb1_t[:], fb1_d[:])
        fb2_t = consts.tile([128, NB * C], F32)
        nc.sync.dma_start(fb2_t[:], fb2_d[:])
        ow_t = consts.tile([128, C, V], BF16)
        nc.sync.dma_start(ow_t[:], ow_d[:].rearrange("p (c v) -> p c v", v=V))
        ob_t = consts.tile([V, 1], F32)
        nc.sync.dma_start(ob_t[:], ob_d[:])
        lng_t = lnb_t = None
        if ln_general_params:
            lng_t = consts.tile([128, 2 * NB + 1, C], F32)
            nc.sync.dma_start(lng_t[:], lng_d[:].rearrange(
                "p (l c) -> p l c", c=C))
            lnb_t = consts.tile([128, 2 * NB + 1, C], F32)
            nc.sync.dma_start(lnb_t[:], lnb_d[:].rearrange(
                "p (l c) -> p l c", c=C))

        h_t = hpool.tile([128, C, NTOK], F32)
        nc.sync.dma_start(h_t[:], h0_d[:].rearrange(
            "p (c t) -> p c t", t=NTOK))

        def ln_params(idx):
            if ln_general_params and not ln_trivial[idx]:
                return lng_t[:, idx, :], lnb_t[:, idx, :], False
            return None, None, True

        for i in range(nb_run):
            # ---- load this layer's weights ----
            wq_t = wqkv.tile([128, C, 512], BF16, tag="wq")
            nc.sync.dma_start(wq_t[:], wq_d[i].rearrange(
                "p (c m) -> p c m", m=512))
            wk_t = wqkv.tile([128, C, 512], BF16, tag="wk")
            nc.sync.dma_start(wk_t[:], wk_d[i].rearrange(
                "p (c m) -> p c m", m=512))
            wv_t = wqkv.tile([128, C, 512], BF16, tag="wv")
            nc.sync.dma_start(wv_t[:], wv_d[i].rearrange(
                "p (c m) -> p c m", m=512))
            pw_t = wqkv.tile([128, C, 512], BF16, tag="pw")
            nc.sync.dma_start(pw_t[:], pw_d[i].rearrange(
                "p (c m) -> p c m", m=512))
            f1_t = wff1.tile([128, C, FF], BF16, tag="f1")
            nc.sync.dma_start(f1_t[:], f1_d[i].rearrange(
                "p (c m) -> p c m", m=FF))
            f2_t = wff2.tile([128, CF, 512], BF16, tag="f2")
            nc.sync.dma_start(f2_t[:], f2_d[i].rearrange(
                "p (c m) -> p c m", m=512))

            # ---- LN1 (layer 0: pre-peeled below; others peeled into
            #      the previous layer's FFN emission) ----
            if i == 0:
                g_ap, b_ap, triv = ln_params(0)
                xn = _emit_ln(nc, pools, h_t, ones_t, eps2_t, g_ap, b_ap,
                              triv)
            else:
                xn = xn_next

            # ---- V projection, token-major: vT[tk, hd*64+d] ----
            vt_t = vt.tile([128, SEQ * NJ, 512], BF16, tag="vt")
            for jg in range(SEQ * NJ):
                vp = ps1.tile([128, 512], F32, tag="ps1")
                for c in range(C):
                    nc.tensor.matmul(vp[:], xn[:, c, jg * 128:(jg + 1) * 128],
                                     wv_t[:, c, :],
                                     start=(c == 0), stop=(c == C - 1))
                nc.scalar.copy(vt_t[:, jg, :], vp[:])

            o_t = osb.tile([128, C, NTOK], BF16, tag="o")

            def emit_den_o(s, hp, p_t):
                base = s * T
                rdens = []
                for h2 in range(2):
                    den = ps1.tile([128, 512], F32, tag="ps1")
                    for j in range(NJ):
                        off = j * 128
                        njw = T - off
                        nc.tensor.matmul(den[:, off:512], ones_t[:],
                                         p_t[:, h2, j, 0:njw],
                                         start=(j == 0), stop=(j == NJ - 1))
                    rd = stats.tile([128, 512], F32, tag="stats")
                    nc.vector.reciprocal_approx_fast(out=rd[:], in_=den[:])
                    rdens.append(rd)
                op_ps = ps1.tile([128, 512], F32, tag="ps1")
                for h2 in range(2):
                    head = hp * 2 + h2
                    for j in range(NJ):
                        off = j * 128
                        njw = T - off
                        nc.tensor.matmul(
                            op_ps[h2 * 64:h2 * 64 + 64, off:T],
                            vt_t[:, s * NJ + j, head * 64:head * 64 + 64],
                            p_t[:, h2, j, 0:njw],
                            start=(j == 0), stop=(j == NJ - 1))
                for h2 in range(2):
                    dsl = slice(h2 * 64, h2 * 64 + 64)
                    nc.vector.tensor_tensor(
                        o_t[dsl, hp, base:base + T], op_ps[dsl, 0:T],
                        rdens[h2][dsl, :], OP.mult)

            pending = None
            for hp in range(HP):
                msl = slice(hp * 128, (hp + 1) * 128)
                q_t = qk.tile([128, NTOK], BF16, tag="q")
                k_t = qk.tile([128, NTOK], BF16, tag="k")
                for tt in range(2):
                    sl = slice(tt * 512, tt * 512 + 512)
                    qp = ps1.tile([128, 512], F32, tag="ps1")
                    kp = ps1.tile([128, 512], F32, tag="ps1")
                    for c in range(C):
                        nc.tensor.matmul(qp[:], wq_t[:, c, msl],
                                         xn[:, c, sl],
                                         start=(c == 0), stop=(c == C - 1))
                        nc.tensor.matmul(kp[:], wk_t[:, c, msl],
                                         xn[:, c, sl],
                                         start=(c == 0), stop=(c == C - 1))
                    nc.vector.tensor_copy(q_t[:, sl], qp[:])
                    nc.vector.tensor_copy(k_t[:, sl], kp[:])

                for s in range(SEQ):
                    base = s * T
                    p_t = pp.tile([128, 2, NJ, 512], BF16, tag="p")
                    for j in range(NJ):
                        off = j * 128
                        njw = T - off
                        for h2 in range(2):
                            dsl = slice(h2 * 64, h2 * 64 + 64)
                            sT = ps1.tile([128, 512], F32, tag="ps1")
                            nc.tensor.matmul(
                                sT[:, 0:njw],
                                k_t[dsl, base + off:base + off + 128],
                                q_t[dsl, base + off:base + T],
                                start=True, stop=True)
                            nc.scalar.activation(
                                p_t[:, h2, j, 0:njw], sT[:, 0:njw],
                                AF.Exp, scale=SCALE)
                        nc.vector.tensor_tensor(
                            p_t[:, :, j, 0:128], p_t[:, :, j, 0:128],
                            tri_t[:, None, :].to_broadcast(
                                (128, 2, 128)), OP.mult)
                    if pending is not None:
                        emit_den_o(*pending)
                    pending = (s, hp, p_t)
            emit_den_o(*pending)

            # ---- attention out projection + residual ----
            for tt in range(2):
                sl = slice(tt * 512, tt * 512 + 512)
                for mc in range(C):
                    pj = ps1.tile([128, 512], F32, tag="ps1")
                    for c in range(C):
                        nc.tensor.matmul(pj[:],
                                         pw_t[:, c, mc * 128:(mc + 1) * 128],
                                         o_t[:, c, sl],
                                         start=(c == 0), stop=(c == C - 1))
                    nc.vector.scalar_tensor_tensor(
                        out=h_t[:, mc, sl], in0=pj[:],
                        scalar=pb_t[:, i * C + mc:i * C + mc + 1],
                        in1=h_t[:, mc, sl], op0=OP.add, op1=OP.add)

            # ---- LN2 + FFN (token-tile split) ----
            g_ap, b_ap, triv = ln_params(2 * i + 1)
            xn2 = _emit_ln(nc, pools, h_t, ones_t, eps2_t, g_ap, b_ap, triv)

            for tt in range(2):
                sl = slice(tt * 512, tt * 512 + 512)
                fa = ffa.tile([128, CF, 512], BF16, tag="fa")
                for mf in range(CF):
                    fp = ps1.tile([128, 512], F32, tag="ps1")
                    for c in range(C):
                        nc.tensor.matmul(fp[:],
                                         f1_t[:, c, mf * 128:(mf + 1) * 128],
                                         xn2[:, c, sl],
                                         start=(c == 0), stop=(c == C - 1))
                    nc.scalar.activation(
                        fa[:, mf, :], fp[:], AF.Relu,
                        bias=fb1_t[:, i * CF + mf:i * CF + mf + 1])
                for mc in range(C):
                    f2p = ps1.tile([128, 512], F32, tag="ps1")
                    for c16 in range(CF):
                        nc.tensor.matmul(f2p[:],
                                         f2_t[:, c16, mc * 128:(mc + 1) * 128],
                                         fa[:, c16, :],
                                         start=(c16 == 0),
                                         stop=(c16 == CF - 1))
                    nc.vector.scalar_tensor_tensor(
                        out=h_t[:, mc, sl], in0=f2p[:],
                        scalar=fb2_t[:, i * C + mc:i * C + mc + 1],
                        in1=h_t[:, mc, sl], op0=OP.add, op1=OP.add)
                # peel next layer's LN1(tt) here so its scalar/vector chain
                # hides behind the other token-tile's FFN matmuls
                if i + 1 < nb_run:
                    if tt == 0:
                        ln_next = _alloc_ln(pools)
                    g_ap, b_ap, triv = ln_params(2 * (i + 1))
                    _emit_ln_tt(nc, pools, h_t, ones_t, eps2_t, g_ap, b_ap,
                                triv, tt, *ln_next)
                    if tt == 1:
                        xn_next = ln_next[2]

        # ---- final LN + logits ----
        g_ap, b_ap, triv = (ln_params(2 * NB) if nb_run == NB
                            else (None, None, True))
        xnf = _emit_ln(nc, pools, h_t, ones_t, eps2_t, g_ap, b_ap, triv)
        lg_sb = consts.tile([V, NTOK], F32)
        for tt in range(2):
            sl = slice(tt * 512, tt * 512 + 512)
            lg = ps1.tile([V, 512], F32, tag="ps1")
            for c in range(C):
                nc.tensor.matmul(lg[:], ow_t[:, c, :], xnf[:, c, sl],
                                 start=(c == 0), stop=(c == C - 1))
            nc.vector.tensor_scalar_add(lg_sb[:, sl], lg[:], ob_t[:])
        nc.sync.dma_start(out_d[:], lg_sb[:])

    nc.finalize()
    return nc


def prepare_inputs(inputs):
    """Host-side preprocessing: embedding gather, weight layout + bf16 cast.
    Returns (shared_map, per_core_h0_list, ln_trivial)."""
    f32 = np.float32
    bf16 = ml_dtypes.bfloat16
    x = np.asarray(inputs["x"]).astype(np.int64)
    emb = np.asarray(inputs["emb"], dtype=f32)
    pos = np.asarray(inputs["pos"], dtype=f32)

    positions = np.minimum(np.arange(T), L - 1)
    h0 = emb[x] + pos[positions][None, :, :]      # [B, T, E] fp32

    def to_dev_lhst(mat, kchunks, mcols):
        m = np.ascontiguousarray(mat.astype(bf16))
        return m.reshape(kchunks, 128, mcols).transpose(1, 0, 2).reshape(
            128, kchunks * mcols)

    wq = np.asarray(inputs["wq"], dtype=f32)
    wk = np.asarray(inputs["wk"], dtype=f32)
    wv = np.asarray(inputs["wv"], dtype=f32)
    pw = np.asarray(inputs["proj_w"], dtype=f32)
    f1 = np.asarray(inputs["ff_w1"], dtype=f32)
    f2 = np.asarray(inputs["ff_w2"], dtype=f32)

    wq_dev = np.stack([to_dev_lhst(wq[i].transpose(1, 0, 2).reshape(E, NH * HS),
                                   C, 512) for i in range(NB)])
    wk_dev = np.stack([to_dev_lhst(wk[i].transpose(1, 0, 2).reshape(E, NH * HS),
                                   C, 512) for i in range(NB)])
    wv_dev = np.stack([to_dev_lhst(wv[i].transpose(1, 0, 2).reshape(E, NH * HS),
                                   C, 512) for i in range(NB)])
    pw_dev = np.stack([to_dev_lhst(pw[i], C, 512) for i in range(NB)])
    f1_dev = np.stack([to_dev_lhst(f1[i], C, FF) for i in range(NB)])
    f2_dev = np.stack([to_dev_lhst(f2[i], CF, 512) for i in range(NB)])

    def vec_dev(v, chunks):
        return np.ascontiguousarray(v.astype(f32).reshape(chunks, 128).T)

    pb_dev = np.concatenate([vec_dev(np.asarray(inputs["proj_b"][i]), C)
                             for i in range(NB)], axis=1)
    fb1_dev = np.concatenate([vec_dev(np.asarray(inputs["ff_b1"][i]), CF)
                              for i in range(NB)], axis=1)
    fb2_dev = np.concatenate([vec_dev(np.asarray(inputs["ff_b2"][i]), C)
                              for i in range(NB)], axis=1)
    ow_dev = to_dev_lhst(np.asarray(inputs["out_w"], dtype=f32) / TEMP, C, V)
    ob_dev = (np.asarray(inputs["out_b"], dtype=f32) / TEMP).reshape(V, 1)
    tri_dev = np.triu(np.ones((128, 128), dtype=f32)).astype(bf16)

    gs, bs, ln_trivial = [], [], []
    for i in range(NB):
        for nm_g, nm_b in (("ln1_g", "ln1_b"), ("ln2_g", "ln2_b")):
            g = np.asarray(inputs[nm_g][i], dtype=f32)
            b = np.asarray(inputs[nm_b][i], dtype=f32)
            gs.append(vec_dev(g, C))
            bs.append(vec_dev(b, C))
            ln_trivial.append(bool(np.all(g == 1.0) and np.all(b == 0.0)))
    g = np.asarray(inputs["lnf_g"], dtype=f32)
    b = np.asarray(inputs["lnf_b"], dtype=f32)
    gs.append(vec_dev(g, C))
    bs.append(vec_dev(b, C))
    ln_trivial.append(bool(np.all(g == 1.0) and np.all(b == 0.0)))
    lng_dev = np.concatenate(gs, axis=1)
    lnb_dev = np.concatenate(bs, axis=1)

    shared = {
        "wq": wq_dev, "wk": wk_dev, "wv": wv_dev, "pw": pw_dev,
        "f1": f1_dev, "f2": f2_dev, "pb": pb_dev, "fb1": fb1_dev,
        "fb2": fb2_dev, "ow": ow_dev, "ob": ob_dev, "tri": tri_dev,
        "lng": lng_dev, "lnb": lnb_dev,
    }

    h0_cores = []
    for core in range(NCORES):
        hh = h0[SEQ * core:SEQ * core + SEQ]          # [SEQ, T, E]
        hT = hh.transpose(2, 0, 1).reshape(E, NTOK)   # [E, NTOK]
        h0_cores.append(np.ascontiguousarray(
            hT.reshape(C, 128, NTOK).transpose(1, 0, 2).reshape(
                128, C * NTOK)))
    return shared, h0_cores, ln_trivial


def assemble_output(core_logits):
    """core_logits: list of [V, NTOK] fp32 -> [B, T, V]."""
    out = np.empty((B, T, V), np.float32)
    for core in range(NCORES):
        lg = core_logits[core].reshape(V, SEQ, T)
        out[SEQ * core:SEQ * core + SEQ] = lg.transpose(1, 2, 0)
    return out


def get_program(ln_trivial):
    key = tuple(ln_trivial)
    if key not in _PROGRAM_CACHE:
        _PROGRAM_CACHE[key] = build_program(list(key))
    return _PROGRAM_CACHE[key]


def reset_device():
    """Recover a wedged accelerator (axon session reset). Best-effort."""
    try:
        import ctypes
        import jax
        jax.devices()
        lib = ctypes.CDLL('/opt/axon/libaxon_pjrt.so')
        lib.axon_reset.restype = ctypes.c_int64
        lib.axon_reset()
    except Exception:
        pass


def kernel(**inputs):
    from concourse.bass_utils import run_bass_kernel_spmd
    shared, h0_cores, ln_trivial = prepare_inputs(inputs)
    nc = get_program(ln_trivial)
    in_maps = [dict(shared, h0=h0_cores[c]) for c in range(NCORES)]
    try:
        res = run_bass_kernel_spmd(nc, in_maps, core_ids=list(range(NCORES)))
    except Exception:
        # A previous (profiled) session can leave the device wedged; reset
        # the axon session and retry once.
        reset_device()
        res = run_bass_kernel_spmd(nc, in_maps, core_ids=list(range(NCORES)))
    return assemble_output([res.results[c]["logits"] for c in range(NCORES)])
